# revision 38
# baseline (speedup 1.0000x reference)
"""Trainium2 Bass kernel for a 3D non-local attention block.

Math (per batch b):
  xf = x.reshape(C, N)                         C=64, N=32768 (=32^3)
  theta = w_theta @ xf                         [8, N]
  phi   = maxpool2(w_phi @ xf)                 [8, M], M=4096
  g     = maxpool2(w_g   @ xf)                 [32, M]
  beta  = softmax_over_m(theta^T phi)          [N, M]
  o     = g @ beta^T                           [32, N]
  out   = gamma * (w_o @ o) + xf               [C, N]

Sharding: 8 cores, core k -> batch k//4, query slice k%4 (8192 queries).
Every core re-computes the (cheap) pooled phi/g from the full batch and
runs flash-style attention over its own query slice; no collectives.

On-device layout: scores are produced transposed [m(part), n(free)] so
exp runs on ScalarE straight out of PSUM and the second matmul consumes
exp(S) with no transposes in the hot loop; the softmax denominator falls
out of the same matmul as a 33rd row (ones column appended to g^T).
The S matmuls are 3x row-tiled (K=8 zero-padded to 32-row PE tiles at
partition offsets 0/32/64); the o matmuls are 2x column-tiled (even
chunks -> PSUM partitions 0:64, odd -> 64:128 of a second bank that
doubles as the projection output).
"""

import os
import sys

sys.path.insert(0, "/opt/trn_rl_repo")

import numpy as np

C = 64            # channels
N = 32768         # voxels (32^3)
NS = N // 4       # query slice per core (8192)
M = N // 8        # pooled keys (4096)
F = 512           # free-dim tile (PSUM bank)
NT = NS // F      # 16 n-tiles per core
MC = M // 128     # 32 m-chunks of 128
GROUPS = [(s, min(s + 3, MC)) for s in range(0, MC, 3)]  # 3-chunk exp groups


def _build_program(mm_dt_name="float32r"):
    import concourse.bass as bass  # noqa: F401
    import concourse.tile as tile
    from concourse import bacc, mybir
    from concourse.masks import make_identity

    f32 = mybir.dt.float32
    bf16 = mybir.dt.bfloat16
    fp16 = mybir.dt.float16
    mmdt = getattr(mybir.dt, mm_dt_name)

    def mm(ap):
        return ap

    nc = bacc.Bacc()

    x_full = nc.declare_dram_parameter("x_full", [C, N], f32, isOutput=False)
    x_slice = nc.declare_dram_parameter("x_slice", [C, NS], f32, isOutput=False)
    w_pg = nc.declare_dram_parameter("w_pg", [C, 64], f32, isOutput=False)
    w_th = nc.declare_dram_parameter("w_th", [C, 32], f32, isOutput=False)
    w_oT = nc.declare_dram_parameter("w_oT", [32, C], f32, isOutput=False)
    gamma = nc.declare_dram_parameter("gamma", [1, 1], f32, isOutput=False)
    out_d = nc.declare_dram_parameter("out", [C, NS], f32, isOutput=True)

    Exp = mybir.ActivationFunctionType.Exp
    Max = mybir.AluOpType.max
    Add = mybir.AluOpType.add

    with tile.TileContext(nc) as tc:
        with (
            tc.tile_pool(name="consts", bufs=1) as consts,
            tc.tile_pool(name="big", bufs=2) as bigpool,
            tc.tile_pool(name="s1p", bufs=1) as s1pool,
            tc.tile_pool(name="s2p", bufs=1) as s2pool,
            tc.tile_pool(name="theta", bufs=1) as thpool,
            tc.tile_pool(name="pg", bufs=1) as pgpool,
            tc.tile_pool(name="gtp", bufs=1) as gtpool,
            tc.tile_pool(name="xin", bufs=2) as xpool,
            tc.tile_pool(name="small", bufs=2) as smallpool,
            tc.tile_pool(name="outp", bufs=2) as outpool,
        ):
            w_pg_sb = consts.tile([C, 64], mmdt)
            nc.gpsimd.dma_start(out=w_pg_sb, in_=w_pg[:])
            w_th_sb = consts.tile([C, 32], mmdt)
            nc.gpsimd.dma_start(out=w_th_sb, in_=w_th[:])
            w_oT_sb = consts.tile([32, C], mmdt)
            gamma_sb = consts.tile([1, 1], f32)
            nc.sync.dma_start(out=gamma_sb, in_=gamma[:])
            w_oT_f32 = consts.tile([32, C], f32)
            nc.sync.dma_start(out=w_oT_f32, in_=w_oT[:])
            g32 = consts.tile([32, 1], f32)
            nc.gpsimd.partition_broadcast(g32, gamma_sb)
            nc.vector.tensor_scalar_mul(w_oT_sb, w_oT_f32, g32)
            ident = consts.tile([32, 32], bf16)
            make_identity(nc, ident)
            ones32 = consts.tile([128, 32], f32)
            nc.vector.memset(ones32, 1.0)
            zeros_sb = consts.tile([128, F], f32)
            nc.vector.memset(zeros_sb, 0.0)

            # pooled g (w_pg rows 0:32) and phi (rows 32:40); phi carries
            # replicas at partition offsets 32/64 for the row-tiled S.
            phi_sb = pgpool.tile([96, M], fp16)
            g_sb = pgpool.tile([32, M], bf16)

            theta_sb = thpool.tile([96, NS], fp16, tag="th96")

            # G' = [g^T | 1], zero-padded to 64 columns, chunk-major.
            gt = gtpool.tile([128, MC, 64], bf16)
            gtv = gt.rearrange("p a b -> p (a b)")
            for z0 in range(0, MC * 64, F):
                nc.vector.tensor_copy(gtv[:, z0 : z0 + F], zeros_sb[:, 0:F])
            nc.vector.tensor_copy(gt[:, :, 32], ones32)

            # ---- Phase A: fused phi/g projection + 2x2x2 maxpool over four
            # 8192-column quarter slabs (d in [8q, 8q+8)); theta, the phi
            # replicas and this quarter's slice of G' are produced in-line so
            # the attention loop can start consuming chunk 0 immediately.
            with tc.tile_pool(name="psA", bufs=2, space="PSUM") as psA:
                for q in range(4):
                    # stage-1 w-pair pooling is fused into PSUM evacuation:
                    # a single max-reduce over the innermost pair axis.
                    s1 = s1pool.tile([64, 4096], f32)
                    for cch in range(4):  # 2048-col x chunks
                        base = q * 8192 + cch * 2048
                        xc = xpool.tile([C, 2048], mmdt, tag="x", bufs=3)
                        nc.gpsimd.dma_start(out=xc, in_=x_full[:, base : base + 2048])
                        for half in range(2):
                            ps = psA.tile([64, 1024], f32, tag="psA", bufs=2)
                            for k in range(2):
                                kk = half * 2 + k
                                nc.tensor.matmul(
                                    ps[:, k * F : (k + 1) * F],
                                    mm(w_pg_sb),
                                    mm(xc[:, kk * F : (kk + 1) * F]),
                                    start=True,
                                    stop=True,
                                )
                            nc.vector.tensor_reduce(
                                s1[:, cch * 1024 + half * 512 : cch * 1024 + (half + 1) * 512],
                                ps.rearrange("c (m two) -> c m two", two=2),
                                mybir.AxisListType.X,
                                Max,
                            )
                    # pool h-pairs: [40, 8, 16, 2, 16] -> [40, 2048]
                    s2 = s2pool.tile([64, 2048], f32)
                    v = s1.rearrange(
                        "c (d hh two w) -> c d hh two w", d=8, hh=16, two=2, w=16
                    )
                    nc.vector.tensor_tensor(s2, v[:, :, :, 0, :], v[:, :, :, 1, :], Max)
                    # pool d-pairs: [40, 4, 2, 256] -> [40, 1024]
                    v = s2.rearrange("c (d two r) -> c d two r", d=4, two=2, r=256)
                    m0 = q * 1024
                    nc.vector.tensor_tensor(
                        g_sb[:, m0 : m0 + 1024], v[0:32, :, 0, :], v[0:32, :, 1, :], Max
                    )
                    nc.vector.tensor_tensor(
                        phi_sb[0:32, m0 : m0 + 1024],
                        v[32:64, :, 0, :],
                        v[32:64, :, 1, :],
                        Max,
                    )
                    # replicate this quarter's phi to partition offsets 32/64
                    for off in (32, 64):
                        nc.sync.dma_start(
                            out=phi_sb[off : off + 32, m0 : m0 + 1024],
                            in_=phi_sb[0:32, m0 : m0 + 1024],
                        )

                    # theta projection for slice chunk q
                    xt = xpool.tile([C, 2048], mmdt, tag="x", bufs=3)
                    nc.gpsimd.dma_start(
                        out=xt, in_=x_slice[:, q * 2048 : (q + 1) * 2048]
                    )
                    for half in range(2):
                        ps = psA.tile([32, 1024], f32, tag="psTh", bufs=1)
                        for k in range(2):
                            kk = half * 2 + k
                            nc.tensor.matmul(
                                ps[:, k * F : (k + 1) * F],
                                mm(w_th_sb),
                                mm(xt[:, kk * F : (kk + 1) * F]),
                                start=True,
                                stop=True,
                            )
                        nc.vector.tensor_copy(
                            theta_sb[
                                0:32,
                                q * 2048 + half * 1024 : q * 2048 + (half + 1) * 1024,
                            ],
                            ps,
                        )
                    for off in (32, 64):
                        nc.sync.dma_start(
                            out=theta_sb[off : off + 32, q * 2048 : (q + 1) * 2048],
                            in_=theta_sb[0:32, q * 2048 : (q + 1) * 2048],
                        )

                    # this quarter's slice of G' (8 transposed chunks)
                    for j in range(8 * q, 8 * q + 8):
                        tps = psA.tile([128, 32], bf16, tag="psB", bufs=2)
                        nc.tensor.transpose(
                            tps, g_sb[:, j * 128 : (j + 1) * 128], ident
                        )
                        nc.vector.tensor_copy(gt[:, j, 0:32], tps)

            # ---- Phase C: flash attention, software-pipelined across the
            # 16 n-tiles: exp groups stream on ScalarE; o matmuls consume
            # exp(S) as soon as each group lands (even/odd column tiles run
            # concurrently); each tile's normalize/project/store tail is
            # deferred into the next tile's groups so ScalarE never drains.
            with (
                tc.tile_pool(name="psS", bufs=2, space="PSUM") as psS,
                tc.tile_pool(name="psO", bufs=1, space="PSUM") as psO_p,
                tc.tile_pool(name="psP", bufs=1, space="PSUM") as psP,
            ):
                def emit_o(st, mc):
                    par = mc % 2
                    nc.tensor.matmul(
                        st["psO"][0:64, :] if par == 0 else st["po2"][64:128, :],
                        gt[:, mc, :],
                        st["expS"][:, mc, :],
                        start=(mc < 2),
                        stop=(mc >= MC - 2),
                        tile_position=(0, 0) if par == 0 else (0, 64),
                    )

                def emit_tail(st):
                    n0 = st["n0"]
                    psO, po2 = st["psO"], st["po2"]
                    o_b = smallpool.tile([33, F], f32, tag="ob", bufs=1)
                    nc.vector.tensor_copy(o_b, po2[64:97, :])
                    o_m = smallpool.tile([32, F], f32, tag="om", bufs=1)
                    nc.vector.tensor_tensor(o_m, psO[0:32, :], o_b[0:32, :], Add)
                    den = smallpool.tile([1, F], f32, tag="den")
                    nc.vector.tensor_tensor(den, psO[32:33, :], o_b[32:33, :], Add)
                    nc.vector.reciprocal_approx_fast(out=den, in_=den)
                    rb = smallpool.tile([32, F], f32, tag="rb")
                    nc.gpsimd.partition_broadcast(rb, den)
                    o_sb = smallpool.tile([32, F], mmdt, tag="osb", bufs=1)
                    nc.vector.tensor_mul(o_sb, o_m, rb)
                    nc.tensor.matmul(
                        po2[0:64, :], mm(w_oT_sb), mm(o_sb), start=True, stop=True
                    )
                    xres = xpool.tile([C, F], f32, tag="xres")
                    nc.sync.dma_start(out=xres, in_=x_slice[:, n0 : n0 + F])
                    ot = outpool.tile([C, F], f32)
                    nc.vector.tensor_add(ot, po2[0:64, :], xres)
                    nc.sync.dma_start(out=out_d[:, n0 : n0 + F], in_=ot)

                def make_state(t):
                    return {
                        "n0": t * F,
                        "expS": bigpool.tile([128, MC, F], bf16, tag="big", name="expS"),
                        "psO": psO_p.tile([128, F], f32, name="psO"),
                        "po2": psP.tile([128, F], f32, name="po2"),
                        "ready": 0,
                        "odone": 0,
                    }

                def emit_group(st, gi):
                    mc0, mc1 = GROUPS[gi]
                    cnt = mc1 - mc0
                    sps = psS.tile([128, 3 * F], f32, tag="psS", name="sps")
                    for i, mc in enumerate(range(mc0, mc1)):
                        nc.tensor.matmul(
                            sps[:, i * F : (i + 1) * F],
                            mm(phi_sb[32 * i : 32 * i + 8, mc * 128 : (mc + 1) * 128]),
                            mm(theta_sb[32 * i : 32 * i + 8, st["n0"] : st["n0"] + F]),
                            start=True,
                            stop=True,
                            tile_position=(32 * i, 0),
                        )
                    nc.scalar.activation(
                        out=st["expS"][:, mc0:mc1, :], in_=sps[:, 0 : cnt * F], func=Exp
                    )
                    st["ready"] = mc1

                NG = len(GROUPS)
                st = make_state(0)
                start_gi = 0
                for t in range(NT):
                    nxt = None
                    for gi in range(start_gi, NG):
                        emit_group(st, gi)
                        if t + 1 < NT:
                            # pre-emit the next tile's first groups so the
                            # exp stream rides over this tile's o-drain/tail
                            if gi == NG - 2:
                                nxt = make_state(t + 1)
                                emit_group(nxt, 0)
                            elif gi == NG - 1:
                                emit_group(nxt, 1)
                        while st["odone"] < st["ready"] - 3:
                            emit_o(st, st["odone"])
                            st["odone"] += 1
                    if nxt is not None:
                        emit_group(nxt, 2)
                    while st["odone"] < MC:
                        emit_o(st, st["odone"])
                        st["odone"] += 1
                    emit_tail(st)
                    st = nxt
                    start_gi = 3

    nc.finalize()
    return nc


def _maybe_trace_setup():
    """Optional NTFF profiling (test harness only, via NLATTN_TRACE=1)."""
    if not os.environ.get("NLATTN_TRACE"):
        return False
    import types

    try:
        from antenv.axon_hooks import get_axon_ntff_profile_hook  # noqa: F401
    except ImportError:
        import antenv

        mod = types.ModuleType("antenv.axon_hooks")
        mod._hook = None

        def set_axon_ntff_profile_hook(h):
            mod._hook = h

        def get_axon_ntff_profile_hook():
            return mod._hook

        mod.set_axon_ntff_profile_hook = set_axon_ntff_profile_hook
        mod.get_axon_ntff_profile_hook = get_axon_ntff_profile_hook
        sys.modules["antenv.axon_hooks"] = mod
        antenv.axon_hooks = mod
        from trn_agent_boot.trn_boot import _ntff_profile_via_ctypes

        mod._hook = _ntff_profile_via_ctypes("/opt/axon/libaxon_pjrt.so")
    import concourse.bass_utils as bu

    bu.upload_artifacts = lambda tmpdir: "local://" + str(tmpdir)
    return True


_LAST_RESULT = {}


def kernel(x, w_theta, w_phi, w_g, w_o, gamma):
    from concourse.bass_utils import run_bass_kernel_spmd

    trace = _maybe_trace_setup()

    B = np.asarray(x).shape[0]
    xf = np.ascontiguousarray(np.asarray(x).reshape(B, C, N), dtype=np.float32)
    w_pg_h = np.ascontiguousarray(
        np.concatenate(
            [np.asarray(w_g), np.asarray(w_phi), np.zeros((24, C), np.float32)],
            axis=0,
        ).T,
        dtype=np.float32,
    )
    w_th_h = np.ascontiguousarray(
        np.concatenate([np.asarray(w_theta), np.zeros((24, C), np.float32)], axis=0).T,
        dtype=np.float32,
    )
    w_oT_h = np.ascontiguousarray(np.asarray(w_o).T, dtype=np.float32)
    gamma_h = np.asarray(gamma, dtype=np.float32).reshape(1, 1)

    nc = _build_program(os.environ.get("NLATTN_MM_DT", "float32r"))

    in_maps = []
    for core in range(8):
        b, s = core // 4, core % 4
        in_maps.append(
            {
                "x_full": xf[b],
                "x_slice": np.ascontiguousarray(xf[b][:, s * NS : (s + 1) * NS]),
                "w_pg": w_pg_h,
                "w_th": w_th_h,
                "w_oT": w_oT_h,
                "gamma": gamma_h,
            }
        )

    res = run_bass_kernel_spmd(nc, in_maps, core_ids=list(range(8)), trace=trace)
    _LAST_RESULT["exec_time_ns"] = res.exec_time_ns
    _LAST_RESULT["trace"] = res.instructions_and_trace

    out = np.empty((B, C, N), dtype=np.float32)
    for core in range(8):
        b, s = core // 4, core % 4
        out[b][:, s * NS : (s + 1) * NS] = res.results[core]["out"]
    D = H = W = 32
    return out.reshape(B, C, D, H, W)


# revision 39
# speedup vs baseline: 1.0958x; 1.0958x over previous
"""Trainium2 Bass kernel for a 3D non-local attention block.

Math (per batch b):
  xf = x.reshape(C, N)                         C=64, N=32768 (=32^3)
  theta = w_theta @ xf                         [8, N]
  phi   = maxpool2(w_phi @ xf)                 [8, M], M=4096
  g     = maxpool2(w_g   @ xf)                 [32, M]
  beta  = softmax_over_m(theta^T phi)          [N, M]
  o     = g @ beta^T                           [32, N]
  out   = gamma * (w_o @ o) + xf               [C, N]

Sharding: 8 cores, core k -> batch k//4, query slice k%4 (8192 queries).
Every core re-computes the (cheap) pooled phi/g from the full batch and
runs flash-style attention over its own query slice; no collectives.

On-device layout: scores are produced transposed [m(part), n(free)] so
exp runs on ScalarE straight out of PSUM and the second matmul consumes
exp(S) with no transposes in the hot loop; the softmax denominator falls
out of the same matmul as a 33rd row (ones column appended to g^T).
The S matmuls are 3x row-tiled (K=8 zero-padded to 32-row PE tiles at
partition offsets 0/32/64); the o matmuls are 2x column-tiled (even
chunks -> PSUM partitions 0:64, odd -> 64:128 of a second bank that
doubles as the projection output).
"""

import os
import sys

sys.path.insert(0, "/opt/trn_rl_repo")

import numpy as np

C = 64            # channels
N = 32768         # voxels (32^3)
NS = N // 4       # query slice per core (8192)
M = N // 8        # pooled keys (4096)
F = 512           # free-dim tile (PSUM bank)
NT = NS // F      # 16 n-tiles per core
MC = M // 128     # 32 m-chunks of 128
GROUPS = [(s, min(s + 3, MC)) for s in range(0, MC, 3)]  # 3-chunk exp groups


def _build_program(mm_dt_name="float32r"):
    import concourse.bass as bass  # noqa: F401
    import concourse.tile as tile
    from concourse import bacc, mybir
    from concourse.masks import make_identity

    f32 = mybir.dt.float32
    bf16 = mybir.dt.bfloat16
    fp16 = mybir.dt.float16
    mmdt = getattr(mybir.dt, mm_dt_name)

    def mm(ap):
        return ap

    nc = bacc.Bacc()

    x_full = nc.declare_dram_parameter("x_full", [C, N], f32, isOutput=False)
    x_slice = nc.declare_dram_parameter("x_slice", [C, NS], f32, isOutput=False)
    w_pg = nc.declare_dram_parameter("w_pg", [C, 64], f32, isOutput=False)
    w_th = nc.declare_dram_parameter("w_th", [C, 32], f32, isOutput=False)
    w_oT = nc.declare_dram_parameter("w_oT", [32, C], f32, isOutput=False)
    gamma = nc.declare_dram_parameter("gamma", [1, 1], f32, isOutput=False)
    out_d = nc.declare_dram_parameter("out", [C, NS], f32, isOutput=True)

    Exp = mybir.ActivationFunctionType.Exp
    Max = mybir.AluOpType.max
    Add = mybir.AluOpType.add

    with tile.TileContext(nc) as tc:
        with (
            tc.tile_pool(name="consts", bufs=1) as consts,
            tc.tile_pool(name="big", bufs=2) as bigpool,
            tc.tile_pool(name="s1p", bufs=1) as s1pool,
            tc.tile_pool(name="s2p", bufs=1) as s2pool,
            tc.tile_pool(name="theta", bufs=1) as thpool,
            tc.tile_pool(name="pg", bufs=1) as pgpool,
            tc.tile_pool(name="gtp", bufs=1) as gtpool,
            tc.tile_pool(name="xin", bufs=2) as xpool,
            tc.tile_pool(name="small", bufs=2) as smallpool,
            tc.tile_pool(name="outp", bufs=2) as outpool,
        ):
            w_pg_sb = consts.tile([C, 64], fp16)
            nc.gpsimd.dma_start(out=w_pg_sb, in_=w_pg[:])
            w_th_sb = consts.tile([C, 32], fp16)
            nc.gpsimd.dma_start(out=w_th_sb, in_=w_th[:])
            w_oT_sb = consts.tile([32, C], mmdt)
            gamma_sb = consts.tile([1, 1], f32)
            nc.sync.dma_start(out=gamma_sb, in_=gamma[:])
            w_oT_f32 = consts.tile([32, C], f32)
            nc.sync.dma_start(out=w_oT_f32, in_=w_oT[:])
            g32 = consts.tile([32, 1], f32)
            nc.gpsimd.partition_broadcast(g32, gamma_sb)
            nc.vector.tensor_scalar_mul(w_oT_sb, w_oT_f32, g32)
            ident = consts.tile([32, 32], bf16)
            make_identity(nc, ident)
            ones32 = consts.tile([128, 32], f32)
            nc.vector.memset(ones32, 1.0)
            zeros_sb = consts.tile([128, F], f32)
            nc.vector.memset(zeros_sb, 0.0)

            # pooled g (w_pg rows 0:32) and phi (rows 32:40); phi carries
            # replicas at partition offsets 32/64 for the row-tiled S.
            phi_sb = pgpool.tile([96, M], fp16)
            g_sb = pgpool.tile([32, M], bf16)

            theta_sb = thpool.tile([96, NS], fp16, tag="th96")

            # G' = [g^T | 1], zero-padded to 64 columns, chunk-major.
            gt = gtpool.tile([128, MC, 64], bf16)
            gtv = gt.rearrange("p a b -> p (a b)")
            for z0 in range(0, MC * 64, F):
                nc.vector.tensor_copy(gtv[:, z0 : z0 + F], zeros_sb[:, 0:F])
            nc.vector.tensor_copy(gt[:, :, 32], ones32)

            # ---- Phase A: fused phi/g projection + 2x2x2 maxpool over four
            # 8192-column quarter slabs (d in [8q, 8q+8)); theta, the phi
            # replicas and this quarter's slice of G' are produced in-line so
            # the attention loop can start consuming chunk 0 immediately.
            with tc.tile_pool(name="psA", bufs=2, space="PSUM") as psA:
                for q in range(4):
                    # stage-1 w-pair pooling is fused into PSUM evacuation:
                    # a single max-reduce over the innermost pair axis.
                    s1 = s1pool.tile([64, 4096], f32)
                    for cch in range(4):  # 2048-col x chunks
                        base = q * 8192 + cch * 2048
                        xc = xpool.tile([C, 2048], fp16, tag="x", bufs=3)
                        nc.gpsimd.dma_start(out=xc, in_=x_full[:, base : base + 2048])
                        for half in range(2):
                            ps = psA.tile([64, 1024], f32, tag="psA", bufs=2)
                            for k in range(2):
                                kk = half * 2 + k
                                nc.tensor.matmul(
                                    ps[:, k * F : (k + 1) * F],
                                    mm(w_pg_sb),
                                    mm(xc[:, kk * F : (kk + 1) * F]),
                                    start=True,
                                    stop=True,
                                )
                            nc.vector.tensor_reduce(
                                s1[:, cch * 1024 + half * 512 : cch * 1024 + (half + 1) * 512],
                                ps.rearrange("c (m two) -> c m two", two=2),
                                mybir.AxisListType.X,
                                Max,
                            )
                    # pool h-pairs: [40, 8, 16, 2, 16] -> [40, 2048]
                    s2 = s2pool.tile([64, 2048], f32)
                    v = s1.rearrange(
                        "c (d hh two w) -> c d hh two w", d=8, hh=16, two=2, w=16
                    )
                    nc.vector.tensor_tensor(s2, v[:, :, :, 0, :], v[:, :, :, 1, :], Max)
                    # pool d-pairs: [40, 4, 2, 256] -> [40, 1024]
                    v = s2.rearrange("c (d two r) -> c d two r", d=4, two=2, r=256)
                    m0 = q * 1024
                    nc.vector.tensor_tensor(
                        g_sb[:, m0 : m0 + 1024], v[0:32, :, 0, :], v[0:32, :, 1, :], Max
                    )
                    nc.vector.tensor_tensor(
                        phi_sb[0:32, m0 : m0 + 1024],
                        v[32:64, :, 0, :],
                        v[32:64, :, 1, :],
                        Max,
                    )
                    # replicate this quarter's phi to partition offsets 32/64
                    for off in (32, 64):
                        nc.sync.dma_start(
                            out=phi_sb[off : off + 32, m0 : m0 + 1024],
                            in_=phi_sb[0:32, m0 : m0 + 1024],
                        )

                    # theta projection for slice chunk q
                    xt = xpool.tile([C, 2048], fp16, tag="x", bufs=3)
                    nc.gpsimd.dma_start(
                        out=xt, in_=x_slice[:, q * 2048 : (q + 1) * 2048]
                    )
                    for half in range(2):
                        ps = psA.tile([32, 1024], f32, tag="psTh", bufs=1)
                        for k in range(2):
                            kk = half * 2 + k
                            nc.tensor.matmul(
                                ps[:, k * F : (k + 1) * F],
                                mm(w_th_sb),
                                mm(xt[:, kk * F : (kk + 1) * F]),
                                start=True,
                                stop=True,
                            )
                        nc.vector.tensor_copy(
                            theta_sb[
                                0:32,
                                q * 2048 + half * 1024 : q * 2048 + (half + 1) * 1024,
                            ],
                            ps,
                        )
                    for off in (32, 64):
                        nc.sync.dma_start(
                            out=theta_sb[off : off + 32, q * 2048 : (q + 1) * 2048],
                            in_=theta_sb[0:32, q * 2048 : (q + 1) * 2048],
                        )

                    # this quarter's slice of G' (8 transposed chunks)
                    for j in range(8 * q, 8 * q + 8):
                        tps = psA.tile([128, 32], bf16, tag="psB", bufs=2)
                        nc.tensor.transpose(
                            tps, g_sb[:, j * 128 : (j + 1) * 128], ident
                        )
                        nc.vector.tensor_copy(gt[:, j, 0:32], tps)

            # ---- Phase C: flash attention, software-pipelined across the
            # 16 n-tiles: exp groups stream on ScalarE; o matmuls consume
            # exp(S) as soon as each group lands (even/odd column tiles run
            # concurrently); each tile's normalize/project/store tail is
            # deferred into the next tile's groups so ScalarE never drains.
            with (
                tc.tile_pool(name="psS", bufs=2, space="PSUM") as psS,
                tc.tile_pool(name="psO", bufs=1, space="PSUM") as psO_p,
                tc.tile_pool(name="psP", bufs=1, space="PSUM") as psP,
            ):
                def emit_o(st, mc):
                    par = mc % 2
                    nc.tensor.matmul(
                        st["psO"][0:64, :] if par == 0 else st["po2"][64:128, :],
                        gt[:, mc, :],
                        st["expS"][:, mc, :],
                        start=(mc < 2),
                        stop=(mc >= MC - 2),
                        tile_position=(0, 0) if par == 0 else (0, 64),
                    )

                def emit_tail(st):
                    # project the UNNORMALIZED o and divide on the output:
                    # w_o @ (o/den) == (w_o @ o)/den, so the reciprocal and
                    # its broadcast run in parallel with the projection.
                    n0 = st["n0"]
                    psO, po2 = st["psO"], st["po2"]
                    o_b = smallpool.tile([33, F], f32, tag="ob", bufs=1)
                    nc.vector.tensor_copy(o_b, po2[64:97, :])
                    o_sb = smallpool.tile([32, F], mmdt, tag="osb", bufs=1)
                    nc.vector.tensor_tensor(o_sb, psO[0:32, :], o_b[0:32, :], Add)
                    den = smallpool.tile([1, F], f32, tag="den")
                    nc.vector.tensor_tensor(den, psO[32:33, :], o_b[32:33, :], Add)
                    nc.vector.reciprocal_approx_fast(out=den, in_=den)
                    rb = smallpool.tile([64, F], f32, tag="rb")
                    nc.gpsimd.partition_broadcast(rb, den)
                    nc.tensor.matmul(
                        po2[0:64, :], mm(w_oT_sb), mm(o_sb), start=True, stop=True
                    )
                    xres = xpool.tile([C, F], f32, tag="xres")
                    nc.sync.dma_start(out=xres, in_=x_slice[:, n0 : n0 + F])
                    pn = smallpool.tile([64, F], f32, tag="pn", bufs=1)
                    nc.vector.tensor_mul(pn, po2[0:64, :], rb)
                    ot = outpool.tile([C, F], f32)
                    nc.vector.tensor_add(ot, pn, xres)
                    nc.sync.dma_start(out=out_d[:, n0 : n0 + F], in_=ot)

                def make_state(t):
                    return {
                        "n0": t * F,
                        "expS": bigpool.tile([128, MC, F], bf16, tag="big", name="expS"),
                        "psO": psO_p.tile([128, F], f32, name="psO"),
                        "po2": psP.tile([128, F], f32, name="po2"),
                        "ready": 0,
                        "odone": 0,
                    }

                def emit_group(st, gi):
                    mc0, mc1 = GROUPS[gi]
                    cnt = mc1 - mc0
                    sps = psS.tile([128, 3 * F], f32, tag="psS", name="sps")
                    for i, mc in enumerate(range(mc0, mc1)):
                        nc.tensor.matmul(
                            sps[:, i * F : (i + 1) * F],
                            mm(phi_sb[32 * i : 32 * i + 8, mc * 128 : (mc + 1) * 128]),
                            mm(theta_sb[32 * i : 32 * i + 8, st["n0"] : st["n0"] + F]),
                            start=True,
                            stop=True,
                            tile_position=(32 * i, 0),
                        )
                    nc.scalar.activation(
                        out=st["expS"][:, mc0:mc1, :], in_=sps[:, 0 : cnt * F], func=Exp
                    )
                    st["ready"] = mc1

                NG = len(GROUPS)
                st = make_state(0)
                start_gi = 0
                for t in range(NT):
                    nxt = None
                    for gi in range(start_gi, NG):
                        emit_group(st, gi)
                        if t + 1 < NT:
                            # pre-emit the next tile's first groups so the
                            # exp stream rides over this tile's o-drain/tail
                            if gi == NG - 2:
                                nxt = make_state(t + 1)
                                emit_group(nxt, 0)
                            elif gi == NG - 1:
                                emit_group(nxt, 1)
                        while st["odone"] < st["ready"] - 3:
                            emit_o(st, st["odone"])
                            st["odone"] += 1
                    if nxt is not None:
                        emit_group(nxt, 2)
                    while st["odone"] < MC:
                        emit_o(st, st["odone"])
                        st["odone"] += 1
                    emit_tail(st)
                    st = nxt
                    start_gi = 3

    nc.finalize()
    return nc


def _maybe_trace_setup():
    """Optional NTFF profiling (test harness only, via NLATTN_TRACE=1)."""
    if not os.environ.get("NLATTN_TRACE"):
        return False
    import types

    try:
        from antenv.axon_hooks import get_axon_ntff_profile_hook  # noqa: F401
    except ImportError:
        import antenv

        mod = types.ModuleType("antenv.axon_hooks")
        mod._hook = None

        def set_axon_ntff_profile_hook(h):
            mod._hook = h

        def get_axon_ntff_profile_hook():
            return mod._hook

        mod.set_axon_ntff_profile_hook = set_axon_ntff_profile_hook
        mod.get_axon_ntff_profile_hook = get_axon_ntff_profile_hook
        sys.modules["antenv.axon_hooks"] = mod
        antenv.axon_hooks = mod
        from trn_agent_boot.trn_boot import _ntff_profile_via_ctypes

        mod._hook = _ntff_profile_via_ctypes("/opt/axon/libaxon_pjrt.so")
    import concourse.bass_utils as bu

    bu.upload_artifacts = lambda tmpdir: "local://" + str(tmpdir)
    return True


_LAST_RESULT = {}


def kernel(x, w_theta, w_phi, w_g, w_o, gamma):
    from concourse.bass_utils import run_bass_kernel_spmd

    trace = _maybe_trace_setup()

    B = np.asarray(x).shape[0]
    xf = np.ascontiguousarray(np.asarray(x).reshape(B, C, N), dtype=np.float32)
    w_pg_h = np.ascontiguousarray(
        np.concatenate(
            [np.asarray(w_g), np.asarray(w_phi), np.zeros((24, C), np.float32)],
            axis=0,
        ).T,
        dtype=np.float32,
    )
    w_th_h = np.ascontiguousarray(
        np.concatenate([np.asarray(w_theta), np.zeros((24, C), np.float32)], axis=0).T,
        dtype=np.float32,
    )
    w_oT_h = np.ascontiguousarray(np.asarray(w_o).T, dtype=np.float32)
    gamma_h = np.asarray(gamma, dtype=np.float32).reshape(1, 1)

    nc = _build_program(os.environ.get("NLATTN_MM_DT", "float32r"))

    in_maps = []
    for core in range(8):
        b, s = core // 4, core % 4
        in_maps.append(
            {
                "x_full": xf[b],
                "x_slice": np.ascontiguousarray(xf[b][:, s * NS : (s + 1) * NS]),
                "w_pg": w_pg_h,
                "w_th": w_th_h,
                "w_oT": w_oT_h,
                "gamma": gamma_h,
            }
        )

    res = run_bass_kernel_spmd(nc, in_maps, core_ids=list(range(8)), trace=trace)
    _LAST_RESULT["exec_time_ns"] = res.exec_time_ns
    _LAST_RESULT["trace"] = res.instructions_and_trace

    out = np.empty((B, C, N), dtype=np.float32)
    for core in range(8):
        b, s = core // 4, core % 4
        out[b][:, s * NS : (s + 1) * NS] = res.results[core]["out"]
    D = H = W = 32
    return out.reshape(B, C, D, H, W)


# revision 40
# speedup vs baseline: 1.1421x; 1.0423x over previous
"""Trainium2 Bass kernel for a 3D non-local attention block.

Math (per batch b):
  xf = x.reshape(C, N)                         C=64, N=32768 (=32^3)
  theta = w_theta @ xf                         [8, N]
  phi   = maxpool2(w_phi @ xf)                 [8, M], M=4096
  g     = maxpool2(w_g   @ xf)                 [32, M]
  beta  = softmax_over_m(theta^T phi)          [N, M]
  o     = g @ beta^T                           [32, N]
  out   = gamma * (w_o @ o) + xf               [C, N]

Sharding: 8 cores, core k -> batch k//4, query slice k%4 (8192 queries).
Every core re-computes the (cheap) pooled phi/g from the full batch and
runs flash-style attention over its own query slice; no collectives.

On-device layout: scores are produced transposed [m(part), n(free)] so
exp runs on ScalarE straight out of PSUM and the second matmul consumes
exp(S) with no transposes in the hot loop; the softmax denominator falls
out of the same matmul as a 33rd row (ones column appended to g^T).
The S matmuls are 3x row-tiled (K=8 zero-padded to 32-row PE tiles at
partition offsets 0/32/64); the o matmuls are 2x column-tiled (even
chunks -> PSUM partitions 0:64, odd -> 64:128 of a second bank that
doubles as the projection output).
"""

import os
import sys

sys.path.insert(0, "/opt/trn_rl_repo")

import numpy as np

C = 64            # channels
N = 32768         # voxels (32^3)
NS = N // 4       # query slice per core (8192)
M = N // 8        # pooled keys (4096)
F = 512           # free-dim tile (PSUM bank)
NT = NS // F      # 16 n-tiles per core
MC = M // 128     # 32 m-chunks of 128
GROUPS = [(s, min(s + 3, MC)) for s in range(0, MC, 3)]  # 3-chunk exp groups


def _build_program(mm_dt_name="float32r"):
    import concourse.bass as bass  # noqa: F401
    import concourse.tile as tile
    from concourse import bacc, mybir
    from concourse.masks import make_identity

    f32 = mybir.dt.float32
    bf16 = mybir.dt.bfloat16
    fp16 = mybir.dt.float16
    mmdt = getattr(mybir.dt, mm_dt_name)

    def mm(ap):
        return ap

    nc = bacc.Bacc()

    x_full = nc.declare_dram_parameter("x_full", [C, N], f32, isOutput=False)
    x_slice = nc.declare_dram_parameter("x_slice", [C, NS], f32, isOutput=False)
    w_pg = nc.declare_dram_parameter("w_pg", [C, 64], f32, isOutput=False)
    w_th = nc.declare_dram_parameter("w_th", [C, 32], f32, isOutput=False)
    w_oT = nc.declare_dram_parameter("w_oT", [32, C], f32, isOutput=False)
    gamma = nc.declare_dram_parameter("gamma", [1, 1], f32, isOutput=False)
    out_d = nc.declare_dram_parameter("out", [C, NS], f32, isOutput=True)

    Exp = mybir.ActivationFunctionType.Exp
    Max = mybir.AluOpType.max
    Add = mybir.AluOpType.add

    with tile.TileContext(nc) as tc:
        with (
            tc.tile_pool(name="consts", bufs=1) as consts,
            tc.tile_pool(name="big", bufs=2) as bigpool,
            tc.tile_pool(name="s1p", bufs=1) as s1pool,
            tc.tile_pool(name="s2p", bufs=1) as s2pool,
            tc.tile_pool(name="theta", bufs=1) as thpool,
            tc.tile_pool(name="pg", bufs=1) as pgpool,
            tc.tile_pool(name="gtp", bufs=1) as gtpool,
            tc.tile_pool(name="xin", bufs=2) as xpool,
            tc.tile_pool(name="small", bufs=2) as smallpool,
            tc.tile_pool(name="outp", bufs=2) as outpool,
        ):
            w_pg_sb = consts.tile([C, 64], fp16)
            nc.gpsimd.dma_start(out=w_pg_sb, in_=w_pg[:])
            w_th_sb = consts.tile([C, 32], fp16)
            nc.gpsimd.dma_start(out=w_th_sb, in_=w_th[:])
            w_oT_sb = consts.tile([32, C], mmdt)
            gamma_sb = consts.tile([1, 1], f32)
            nc.sync.dma_start(out=gamma_sb, in_=gamma[:])
            w_oT_f32 = consts.tile([32, C], f32)
            nc.sync.dma_start(out=w_oT_f32, in_=w_oT[:])
            g32 = consts.tile([32, 1], f32)
            nc.gpsimd.partition_broadcast(g32, gamma_sb)
            nc.vector.tensor_scalar_mul(w_oT_sb, w_oT_f32, g32)
            ident = consts.tile([32, 32], bf16)
            make_identity(nc, ident)
            ones32 = consts.tile([128, 32], f32)
            nc.vector.memset(ones32, 1.0)
            zeros_sb = consts.tile([128, F], f32)
            nc.vector.memset(zeros_sb, 0.0)

            # pooled g (w_pg rows 0:32) and phi (rows 32:40); phi carries
            # replicas at partition offsets 32/64 for the row-tiled S.
            phi_sb = pgpool.tile([96, M], fp16)
            g_sb = pgpool.tile([32, M], bf16)

            theta_sb = thpool.tile([96, NS], fp16, tag="th96")

            # G' = [g^T | 1], zero-padded to 64 columns, chunk-major.
            gt = gtpool.tile([128, MC, 64], bf16)
            gtv = gt.rearrange("p a b -> p (a b)")
            for z0 in range(0, MC * 64, F):
                nc.scalar.copy(gtv[:, z0 : z0 + F], zeros_sb[:, 0:F])
            nc.scalar.copy(gt[:, :, 32], ones32)

            # ---- Phase A: fused phi/g projection + 2x2x2 maxpool over four
            # 8192-column quarter slabs (d in [8q, 8q+8)); theta, the phi
            # replicas and this quarter's slice of G' are produced in-line so
            # the attention loop can start consuming chunk 0 immediately.
            with tc.tile_pool(name="psA", bufs=2, space="PSUM") as psA:
                for q in range(4):
                    # stage-1 w-pair pooling is fused into PSUM evacuation:
                    # a single max-reduce over the innermost pair axis.
                    s1 = s1pool.tile([64, 4096], f32)
                    for cch in range(4):  # 2048-col x chunks
                        base = q * 8192 + cch * 2048
                        xc = xpool.tile([C, 2048], fp16, tag="x", bufs=3)
                        nc.gpsimd.dma_start(out=xc, in_=x_full[:, base : base + 2048])
                        for half in range(2):
                            ps = psA.tile([64, 1024], f32, tag="psA", bufs=2)
                            for k in range(2):
                                kk = half * 2 + k
                                nc.tensor.matmul(
                                    ps[:, k * F : (k + 1) * F],
                                    mm(w_pg_sb),
                                    mm(xc[:, kk * F : (kk + 1) * F]),
                                    start=True,
                                    stop=True,
                                )
                            nc.vector.tensor_reduce(
                                s1[:, cch * 1024 + half * 512 : cch * 1024 + (half + 1) * 512],
                                ps.rearrange("c (m two) -> c m two", two=2),
                                mybir.AxisListType.X,
                                Max,
                            )
                    # pool h-pairs: [40, 8, 16, 2, 16] -> [40, 2048]
                    s2 = s2pool.tile([64, 2048], f32)
                    v = s1.rearrange(
                        "c (d hh two w) -> c d hh two w", d=8, hh=16, two=2, w=16
                    )
                    nc.vector.tensor_tensor(s2, v[:, :, :, 0, :], v[:, :, :, 1, :], Max)
                    # pool d-pairs: [40, 4, 2, 256] -> [40, 1024]
                    v = s2.rearrange("c (d two r) -> c d two r", d=4, two=2, r=256)
                    m0 = q * 1024
                    nc.vector.tensor_tensor(
                        g_sb[:, m0 : m0 + 1024], v[0:32, :, 0, :], v[0:32, :, 1, :], Max
                    )
                    nc.vector.tensor_tensor(
                        phi_sb[0:32, m0 : m0 + 1024],
                        v[32:64, :, 0, :],
                        v[32:64, :, 1, :],
                        Max,
                    )
                    # replicate this quarter's phi to partition offsets 32/64
                    for off in (32, 64):
                        nc.sync.dma_start(
                            out=phi_sb[off : off + 32, m0 : m0 + 1024],
                            in_=phi_sb[0:32, m0 : m0 + 1024],
                        )

                    # theta projection for slice chunk q
                    xt = xpool.tile([C, 2048], fp16, tag="x", bufs=3)
                    nc.gpsimd.dma_start(
                        out=xt, in_=x_slice[:, q * 2048 : (q + 1) * 2048]
                    )
                    for half in range(2):
                        ps = psA.tile([32, 1024], f32, tag="psTh", bufs=1)
                        for k in range(2):
                            kk = half * 2 + k
                            nc.tensor.matmul(
                                ps[:, k * F : (k + 1) * F],
                                mm(w_th_sb),
                                mm(xt[:, kk * F : (kk + 1) * F]),
                                start=True,
                                stop=True,
                            )
                        nc.scalar.copy(
                            theta_sb[
                                0:32,
                                q * 2048 + half * 1024 : q * 2048 + (half + 1) * 1024,
                            ],
                            ps,
                        )
                    for off in (32, 64):
                        nc.sync.dma_start(
                            out=theta_sb[off : off + 32, q * 2048 : (q + 1) * 2048],
                            in_=theta_sb[0:32, q * 2048 : (q + 1) * 2048],
                        )

                    # this quarter's slice of G' (8 transposed chunks)
                    for j in range(8 * q, 8 * q + 8):
                        tps = psA.tile([128, 32], bf16, tag="psB", bufs=2)
                        nc.tensor.transpose(
                            tps, g_sb[:, j * 128 : (j + 1) * 128], ident
                        )
                        nc.scalar.copy(gt[:, j, 0:32], tps)

            # ---- Phase C: flash attention, software-pipelined across the
            # 16 n-tiles: exp groups stream on ScalarE; o matmuls consume
            # exp(S) as soon as each group lands (even/odd column tiles run
            # concurrently); each tile's normalize/project/store tail is
            # deferred into the next tile's groups so ScalarE never drains.
            with (
                tc.tile_pool(name="psS", bufs=2, space="PSUM") as psS,
                tc.tile_pool(name="psO", bufs=1, space="PSUM") as psO_p,
                tc.tile_pool(name="psP", bufs=1, space="PSUM") as psP,
            ):
                def emit_o(st, mc):
                    par = mc % 2
                    nc.tensor.matmul(
                        st["psO"][0:64, :] if par == 0 else st["po2"][64:128, :],
                        gt[:, mc, :],
                        st["expS"][:, mc, :],
                        start=(mc < 2),
                        stop=(mc >= MC - 2),
                        tile_position=(0, 0) if par == 0 else (0, 64),
                    )

                def emit_tail(st):
                    # project the UNNORMALIZED o and divide on the output:
                    # w_o @ (o/den) == (w_o @ o)/den, so the reciprocal and
                    # its broadcast run in parallel with the projection.
                    n0 = st["n0"]
                    psO, po2 = st["psO"], st["po2"]
                    o_b = smallpool.tile([33, F], f32, tag="ob", bufs=1)
                    nc.vector.tensor_copy(o_b, po2[64:97, :])
                    o_sb = smallpool.tile([32, F], mmdt, tag="osb", bufs=1)
                    nc.vector.tensor_tensor(o_sb, psO[0:32, :], o_b[0:32, :], Add)
                    den = smallpool.tile([1, F], f32, tag="den")
                    nc.vector.tensor_tensor(den, psO[32:33, :], o_b[32:33, :], Add)
                    nc.vector.reciprocal_approx_fast(out=den, in_=den)
                    rb = smallpool.tile([64, F], f32, tag="rb")
                    nc.gpsimd.partition_broadcast(rb, den)
                    nc.tensor.matmul(
                        po2[0:64, :], mm(w_oT_sb), mm(o_sb), start=True, stop=True
                    )
                    xres = xpool.tile([C, F], f32, tag="xres")
                    nc.sync.dma_start(out=xres, in_=x_slice[:, n0 : n0 + F])
                    pn = smallpool.tile([64, F], f32, tag="pn", bufs=1)
                    nc.vector.tensor_mul(pn, po2[0:64, :], rb)
                    ot = outpool.tile([C, F], f32)
                    nc.vector.tensor_add(ot, pn, xres)
                    nc.sync.dma_start(out=out_d[:, n0 : n0 + F], in_=ot)

                def make_state(t):
                    return {
                        "n0": t * F,
                        "expS": bigpool.tile([128, MC, F], bf16, tag="big", name="expS"),
                        "psO": psO_p.tile([128, F], f32, name="psO"),
                        "po2": psP.tile([128, F], f32, name="po2"),
                        "ready": 0,
                        "odone": 0,
                    }

                def emit_group(st, gi):
                    mc0, mc1 = GROUPS[gi]
                    cnt = mc1 - mc0
                    sps = psS.tile([128, 3 * F], f32, tag="psS", name="sps")
                    for i, mc in enumerate(range(mc0, mc1)):
                        nc.tensor.matmul(
                            sps[:, i * F : (i + 1) * F],
                            mm(phi_sb[32 * i : 32 * i + 8, mc * 128 : (mc + 1) * 128]),
                            mm(theta_sb[32 * i : 32 * i + 8, st["n0"] : st["n0"] + F]),
                            start=True,
                            stop=True,
                            tile_position=(32 * i, 0),
                        )
                    nc.scalar.activation(
                        out=st["expS"][:, mc0:mc1, :], in_=sps[:, 0 : cnt * F], func=Exp
                    )
                    st["ready"] = mc1

                NG = len(GROUPS)
                st = make_state(0)
                start_gi = 0
                for t in range(NT):
                    nxt = None
                    for gi in range(start_gi, NG):
                        emit_group(st, gi)
                        if t + 1 < NT:
                            # pre-emit the next tile's first groups so the
                            # exp stream rides over this tile's o-drain/tail
                            if gi == NG - 2:
                                nxt = make_state(t + 1)
                                emit_group(nxt, 0)
                            elif gi == NG - 1:
                                emit_group(nxt, 1)
                        while st["odone"] < st["ready"] - 3:
                            emit_o(st, st["odone"])
                            st["odone"] += 1
                    if nxt is not None:
                        emit_group(nxt, 2)
                    while st["odone"] < MC:
                        emit_o(st, st["odone"])
                        st["odone"] += 1
                    emit_tail(st)
                    st = nxt
                    start_gi = 3

    nc.finalize()
    return nc


def _maybe_trace_setup():
    """Optional NTFF profiling (test harness only, via NLATTN_TRACE=1)."""
    if not os.environ.get("NLATTN_TRACE"):
        return False
    import types

    try:
        from antenv.axon_hooks import get_axon_ntff_profile_hook  # noqa: F401
    except ImportError:
        import antenv

        mod = types.ModuleType("antenv.axon_hooks")
        mod._hook = None

        def set_axon_ntff_profile_hook(h):
            mod._hook = h

        def get_axon_ntff_profile_hook():
            return mod._hook

        mod.set_axon_ntff_profile_hook = set_axon_ntff_profile_hook
        mod.get_axon_ntff_profile_hook = get_axon_ntff_profile_hook
        sys.modules["antenv.axon_hooks"] = mod
        antenv.axon_hooks = mod
        from trn_agent_boot.trn_boot import _ntff_profile_via_ctypes

        mod._hook = _ntff_profile_via_ctypes("/opt/axon/libaxon_pjrt.so")
    import concourse.bass_utils as bu

    bu.upload_artifacts = lambda tmpdir: "local://" + str(tmpdir)
    return True


_LAST_RESULT = {}


def kernel(x, w_theta, w_phi, w_g, w_o, gamma):
    from concourse.bass_utils import run_bass_kernel_spmd

    trace = _maybe_trace_setup()

    B = np.asarray(x).shape[0]
    xf = np.ascontiguousarray(np.asarray(x).reshape(B, C, N), dtype=np.float32)
    w_pg_h = np.ascontiguousarray(
        np.concatenate(
            [np.asarray(w_g), np.asarray(w_phi), np.zeros((24, C), np.float32)],
            axis=0,
        ).T,
        dtype=np.float32,
    )
    w_th_h = np.ascontiguousarray(
        np.concatenate([np.asarray(w_theta), np.zeros((24, C), np.float32)], axis=0).T,
        dtype=np.float32,
    )
    w_oT_h = np.ascontiguousarray(np.asarray(w_o).T, dtype=np.float32)
    gamma_h = np.asarray(gamma, dtype=np.float32).reshape(1, 1)

    nc = _build_program(os.environ.get("NLATTN_MM_DT", "float32r"))

    in_maps = []
    for core in range(8):
        b, s = core // 4, core % 4
        in_maps.append(
            {
                "x_full": xf[b],
                "x_slice": np.ascontiguousarray(xf[b][:, s * NS : (s + 1) * NS]),
                "w_pg": w_pg_h,
                "w_th": w_th_h,
                "w_oT": w_oT_h,
                "gamma": gamma_h,
            }
        )

    res = run_bass_kernel_spmd(nc, in_maps, core_ids=list(range(8)), trace=trace)
    _LAST_RESULT["exec_time_ns"] = res.exec_time_ns
    _LAST_RESULT["trace"] = res.instructions_and_trace

    out = np.empty((B, C, N), dtype=np.float32)
    for core in range(8):
        b, s = core // 4, core % 4
        out[b][:, s * NS : (s + 1) * NS] = res.results[core]["out"]
    D = H = W = 32
    return out.reshape(B, C, D, H, W)
